# revision 5
# baseline (speedup 1.0000x reference)
"""Trainium2 kernel for nn_Non_LinearGNN: 8-core SPMD, For_i hardware loops.

Sharding: each core owns 49 contiguous windows of 128 nodes; edges are
sorted by X_Node and packed into BPW blocks of 128 per window (~89% fill).
The per-window segment-sum is exact locally (no AllReduce); one AllGather
publishes H1 for the iter-2 gather, and the final H2 shard is the output.
Node-feature gathers use the core's own aligned shard with local indices
(li + 128*wl); neighbor gathers use the AllGathered global tables.

Program structure: TileContexts with one For_i hardware loop each
(multiple For_i in one context miscompile on this HW path; contexts are
separated by the TileContext drain/semaphore-clear barrier):
  C0  static setup + AllGathers of the feature table / H0 table / weights
  C1  gather loop: per block-pair, 4 indirect DMAs + transpose -> X 2-pack
  C2  Xi MLP loop: 16 fused 64-d layers (block-diag 128x128 weights) -> A
  C3  Rou MLP loop: 11 fused 32/8-d layers (block-diag 4-pack) -> b
  C4  message-passing iter 0 (one-hot scatter matmul) + H1 AllGather
  C5  message-passing iter 1 -> H2 + builds o0=[featT|H2] feature-major
  R1-10 readout BN iterations: chunked f32 MLP + residual, global batch
      stats via a [40,2] AllReduce with exact pad-column correction
  R11 l2 head + 10 l3 tail layers -> o3 [2, VSH] output shard
Host: vectorized (and hash-cached) edge binning only; the readout MLP
runs on device. Why hardware loops: the dominant per-call cost on the
axon-tunneled runtime is shipping/loading the program (~15us per BIR
instruction); For_i bodies are stored once, cutting the NEFF ~10x.
"""

import os
import sys
import time as _time

import numpy as np

sys.path.insert(0, "/opt/trn_rl_repo")

import ml_dtypes

try:
    import jax
    jax.config.update("jax_compilation_cache_dir", "/tmp/jaxcache")
    jax.config.update("jax_persistent_cache_min_entry_size_bytes", -1)
    jax.config.update("jax_persistent_cache_min_compile_time_secs", 0)
except Exception:
    pass

import concourse.bacc as bacc
import concourse.mybir as mybir
import concourse.tile as tile
from concourse import bass, bass_utils
from concourse.masks import make_identity

BF16 = ml_dtypes.bfloat16

V = 50000
E = 400000
LN = 32
S = 8
ITER = 2
DEG = 8.0
MU = 0.8
D = LN + S
EPS = 1e-5
SCALE = MU / S / DEG
NCORES = 8

NW = 392                  # node windows of 128 (covers VP)
NWC = NW // NCORES        # 49 windows per core
VP = NW * 128             # 50176
VSH = VP // NCORES        # 6272 rows per core shard
WR = 336                  # weight-table rows (padded)
WSH = WR // NCORES        # 42

_CACHE = {}
LAST_RESULT = {}


def _build_nc(BPW, PH=99):
    NBLK4 = -(-(NWC * BPW) // 4) * 4        # blocks padded to mult of 4
    NPAIR = NBLK4 // 2
    NQ4 = NBLK4 // 4
    C2 = NBLK4 * 64                          # 2-pack cols
    C2P = -(-C2 // 512) * 512                # padded to 512
    NB2 = C2P // 512
    C4 = NBLK4 * 32                          # 4-pack cols
    C4P = -(-C4 // 512) * 512
    NB4 = C4P // 512
    assert C2P - C2 >= 128 or C2P == C2

    nc = bacc.Bacc("TRN2", target_bir_lowering=False, debug=False)
    dt = mybir.dt
    AF = mybir.ActivationFunctionType
    OP = mybir.AluOpType
    grp = [list(range(NCORES))]

    ft_d = nc.declare_dram_parameter("ftsh", [VSH, 32], dt.bfloat16,
                                     isOutput=False)
    h0_d = nc.declare_dram_parameter("h0sh", [VSH, 8], dt.bfloat16,
                                     isOutput=False)
    w_d = nc.declare_dram_parameter("wsh", [WSH, 64], dt.bfloat16,
                                    isOutput=False)
    li_d = nc.declare_dram_parameter("li8", [128, NBLK4], dt.uint8,
                                     isOutput=False)
    ixe_d = nc.declare_dram_parameter("ixe", [128, NBLK4], dt.uint16,
                                      isOutput=False)
    bx_d = nc.declare_dram_parameter("bx", [128, 8], dt.float32,
                                     isOutput=False)
    wro_d = nc.declare_dram_parameter("wro", [40, 96], dt.float32,
                                      isOutput=False)
    brd_d = nc.declare_dram_parameter("brd", [40, 8], dt.float32,
                                      isOutput=False)
    o3_d = nc.declare_dram_parameter("o3f", [2, VSH], dt.bfloat16,
                                     isOutput=True)

    ftF = nc.dram_tensor("ftF", [VP, 32], dt.bfloat16, kind="Internal",
                         addr_space="Shared")
    h0F = nc.dram_tensor("h0F", [VP, 8], dt.bfloat16, kind="Internal",
                         addr_space="Shared")
    wF = nc.dram_tensor("wF", [WR, 64], dt.bfloat16, kind="Internal",
                        addr_space="Shared")
    H1F = nc.dram_tensor("H1F", [VP, 8], dt.bfloat16, kind="Internal",
                         addr_space="Shared")
    ftL = nc.dram_tensor("ftL", [VSH + 128, 32], dt.bfloat16, kind="Internal")
    ft_b = nc.dram_tensor("ft_b", [VSH, 32], dt.bfloat16, kind="Internal")
    h0_b = nc.dram_tensor("h0_b", [VSH, 8], dt.bfloat16, kind="Internal")
    w_b = nc.dram_tensor("w_b", [WSH, 64], dt.bfloat16, kind="Internal")
    H1d = nc.dram_tensor("H1d", [VSH, 8], dt.bfloat16, kind="Internal")
    xp_d = nc.dram_tensor("xp_d", [128, C2P], dt.bfloat16, kind="Internal")
    xr_d = nc.dram_tensor("xr_d", [128, C4P], dt.bfloat16, kind="Internal")
    ae_d = nc.dram_tensor("ae_d", [128, C2P], dt.bfloat16, kind="Internal")
    be_d = nc.dram_tensor("be_d", [128, C4P // 4], dt.float32, kind="Internal")
    o0_d = nc.dram_tensor("o0_d", [40, VSH], dt.float32, kind="Internal")
    od_d = nc.dram_tensor("od_d", [40, VSH], dt.float32, kind="Internal")
    opad_d = nc.dram_tensor("opad_d", [40, 1], dt.float32, kind="Internal")
    arb_d = nc.dram_tensor("arb_d", [40, 2], dt.float32, kind="Internal")
    arB_d = nc.dram_tensor("arB_d", [40, 2], dt.float32, kind="Internal",
                           addr_space="Shared")

    # ---------------- C0: stage + AllGather tables ----------------
    with tile.TileContext(nc) as tc:
        with tc.tile_pool(name="c0", bufs=1) as p0:
            zt = p0.tile([128, 32], dt.bfloat16)
            nc.vector.memset(zt[:], 0.0)
            nc.sync.dma_start(ft_b[:, :], ft_d[:, :])
            nc.sync.dma_start(ftL[0:VSH, :], ft_d[:, :])
            nc.sync.dma_start(ftL[VSH:VSH + 128, :], zt[:])
            nc.sync.dma_start(h0_b[:, :], h0_d[:, :])
            nc.sync.dma_start(w_b[:, :], w_d[:, :])
            nc.gpsimd.collective_compute(
                "AllGather", OP.bypass, replica_groups=grp,
                ins=[ft_b[:, :].opt()], outs=[ftF[:, :].opt()])
            nc.gpsimd.collective_compute(
                "AllGather", OP.bypass, replica_groups=grp,
                ins=[h0_b[:, :].opt()], outs=[h0F[:, :].opt()])
            nc.gpsimd.collective_compute(
                "AllGather", OP.bypass, replica_groups=grp,
                ins=[w_b[:, :].opt()], outs=[wF[:, :].opt()])

    # ---------------- C1: gather loop -> xp (2-pack X), xr (4-pack) -----
    if PH < 1:
        return _finish_stub(nc, tile, mybir, o3_d, VSH)
    with tile.TileContext(nc) as tc:
        with (
            tc.tile_pool(name="c1r", bufs=1) as res,
            tc.tile_pool(name="c1g", bufs=4) as gp,
            tc.tile_pool(name="c1s", bufs=8) as sp,
            tc.tile_pool(name="c1p", bufs=2, space="PSUM") as pst,
        ):
            li8 = res.tile([128, NBLK4], dt.uint8)
            ixe16 = res.tile([128, NBLK4], dt.uint16)
            ixe32 = res.tile([128, NBLK4], dt.int32)
            ixn32 = res.tile([128, NBLK4], dt.int32)
            wb = res.tile([128, NBLK4], dt.int32)
            ident = res.tile([128, 128], dt.bfloat16)
            xp = res.tile([128, C2P], dt.bfloat16)
            nc.sync.dma_start(li8[:], li_d[:, :])
            nc.sync.dma_start(ixe16[:], ixe_d[:, :])
            nc.vector.tensor_copy(ixe32[:], ixe16[:])
            make_identity(nc, ident[:])
            nc.vector.memset(xp[:], 0.0)
            # local window base per block: 128 * (b // BPW), b < NWC*BPW
            nc.gpsimd.iota(wb[:, 0:NWC * BPW],
                           pattern=[[128, NWC], [0, BPW]],
                           base=0, channel_multiplier=0)
            if NBLK4 > NWC * BPW:
                nc.vector.memset(wb[:, NWC * BPW:], 0.0)
            nc.vector.tensor_copy(ixn32[:], li8[:])
            nc.vector.tensor_tensor(out=ixn32[:], in0=ixn32[:], in1=wb[:],
                                    op=OP.add)
            with tc.For_i(0, NPAIR, 1) as q:
                g = gp.tile([128, 128], dt.bfloat16)
                for k in range(2):
                    stn = sp.tile([128, 1], dt.int32)
                    nc.vector.tensor_copy(stn[:],
                                          ixn32[:, bass.ds(q * 2 + k, 1)])
                    nc.gpsimd.indirect_dma_start(
                        out=g[:, 64 * k:64 * k + 32], out_offset=None,
                        in_=ftL[:, :],
                        in_offset=bass.IndirectOffsetOnAxis(
                            ap=stn[:, 0:1], axis=0))
                    ste = sp.tile([128, 1], dt.int32)
                    nc.vector.tensor_copy(ste[:],
                                          ixe32[:, bass.ds(q * 2 + k, 1)])
                    nc.gpsimd.indirect_dma_start(
                        out=g[:, 64 * k + 32:64 * k + 64], out_offset=None,
                        in_=ftF[:, :],
                        in_offset=bass.IndirectOffsetOnAxis(
                            ap=ste[:, 0:1], axis=0))
                tp = pst.tile([128, 128], dt.bfloat16)
                nc.tensor.transpose(tp[:], g[:], ident[:])
                nc.scalar.activation(xp[:, bass.ts(q, 128)], tp[:], AF.Copy)
            # xr 4-pack: lane a of quad m holds block 4m+a
            xr = res.tile([128, C4P], dt.bfloat16)
            nc.vector.memset(xr[:], 0.0)
            for a in range(4):
                rs = 64 * (a % 2)
                co = 128 * (a // 2)
                nc.vector.tensor_copy(
                    xr[32 * a:32 * a + 32, 0:NQ4 * 128].rearrange(
                        "p (m c) -> p m c", c=128),
                    xp[rs:rs + 32, co:co + NQ4 * 256].rearrange(
                        "p (m c) -> p m c", c=256)[:, :, 0:128])
            nc.sync.dma_start(xp_d[:, :], xp[:])
            nc.sync.dma_start(xr_d[:, :], xr[:])

    # ---------------- C2: Xi MLP loop -> A edge-major -------------------
    if PH < 2:
        return _finish_stub(nc, tile, mybir, o3_d, VSH)
    with tile.TileContext(nc) as tc:
        with (
            tc.tile_pool(name="c2r", bufs=1) as res,
            tc.tile_pool(name="c2a", bufs=2) as ap_,
            tc.tile_pool(name="c2m", bufs=2, space="PSUM") as psm,
            tc.tile_pool(name="c2t", bufs=2, space="PSUM") as pst,
        ):
            xp = res.tile([128, C2P], dt.bfloat16)
            ht = res.tile([128, C2P], dt.bfloat16)
            f2t = res.tile([128, C2P], dt.bfloat16)
            wxi = res.tile([128, 512], dt.bfloat16)
            ident = res.tile([128, 128], dt.bfloat16)
            bx = res.tile([128, 8], dt.float32)
            nc.sync.dma_start(xp[:], xp_d[:, :])
            nc.sync.dma_start(bx[:], bx_d[:, :])
            make_identity(nc, ident[:])
            nc.vector.memset(wxi[:], 0.0)
            for k in range(4):
                nc.sync.dma_start(wxi[0:64, 128 * k:128 * k + 64],
                                  wF[64 * k:64 * k + 64, :])
                nc.sync.dma_start(wxi[64:128, 128 * k + 64:128 * k + 128],
                                  wF[64 * k:64 * k + 64, :])
            with tc.For_i(0, NB2, 1) as i:
                cs = bass.ts(i, 512)
                for l in range(5):
                    src = xp if l == 0 else ht
                    ps = psm.tile([128, 512], dt.float32)
                    nc.tensor.matmul(ps[:], wxi[:, 0:128], src[:, cs],
                                     start=True, stop=False)
                    nc.tensor.matmul(ps[:], ident[:], xp[:, cs],
                                     start=False, stop=True)
                    nc.scalar.activation(ht[:, cs], ps[:], AF.Prelu,
                                         bias=bx[:, 0:1], scale=1.0,
                                         alpha=0.25)
                ps = psm.tile([128, 512], dt.float32)
                nc.tensor.matmul(ps[:], wxi[:, 128:256], ht[:, cs],
                                 start=True, stop=True)
                nc.scalar.activation(f2t[:, cs], ps[:], AF.Prelu,
                                     bias=bx[:, 1:2], scale=1.0, alpha=0.25)
                first = True
                for l in range(5):
                    ps = psm.tile([128, 512], dt.float32)
                    src = f2t if first else ht
                    first = False
                    nc.tensor.matmul(ps[:], wxi[:, 256:384], src[:, cs],
                                     start=True, stop=True)
                    nc.scalar.activation(ht[:, cs], ps[:], AF.Prelu,
                                         bias=bx[:, 2:3], scale=1.0,
                                         alpha=0.25)
                    ps = psm.tile([128, 512], dt.float32)
                    nc.tensor.matmul(ps[:], wxi[:, 384:512], ht[:, cs],
                                     start=True, stop=False)
                    nc.tensor.matmul(ps[:], ident[:], f2t[:, cs],
                                     start=False, stop=True)
                    nc.scalar.activation(ht[:, cs], ps[:], AF.Prelu,
                                         bias=bx[:, 3:4], scale=1.0,
                                         alpha=0.25)
                ast = ap_.tile([128, 512], dt.bfloat16)
                for k in range(4):
                    hstg = ap_.tile([128, 128], dt.bfloat16, tag="hstg")
                    nc.vector.tensor_copy(
                        hstg[:], ht[:, bass.ds(i * 512 + 128 * k, 128)])
                    tp = pst.tile([128, 128], dt.bfloat16)
                    nc.tensor.transpose(tp[:], hstg[:], ident[:])
                    nc.scalar.activation(ast[:, 128 * k:128 * k + 128],
                                         tp[:], AF.Copy)
                nc.sync.dma_start(ae_d[:, cs], ast[:])

    # ---------------- C3: Rou MLP loop -> b edge-major ------------------
    if PH < 3:
        return _finish_stub(nc, tile, mybir, o3_d, VSH)
    with tile.TileContext(nc) as tc:
        with (
            tc.tile_pool(name="c3r", bufs=1) as res,
            tc.tile_pool(name="c3a", bufs=2) as ap_,
            tc.tile_pool(name="c3m", bufs=2, space="PSUM") as psm,
            tc.tile_pool(name="c3t", bufs=2, space="PSUM") as pst,
        ):
            xr = res.tile([128, C4P], dt.bfloat16)
            rh = res.tile([128, C4P], dt.bfloat16)
            rf2 = res.tile([128, C4P], dt.bfloat16)
            wr1 = res.tile([128, 128], dt.bfloat16)
            wr2 = res.tile([128, 32], dt.bfloat16)
            wr3 = res.tile([32, 32], dt.bfloat16)
            ident = res.tile([128, 128], dt.bfloat16)
            bx = res.tile([128, 8], dt.float32)
            nc.sync.dma_start(xr[:], xr_d[:, :])
            nc.sync.dma_start(bx[:], bx_d[:, :])
            make_identity(nc, ident[:])
            nc.vector.memset(wr1[:], 0.0)
            nc.vector.memset(wr2[:], 0.0)
            nc.vector.memset(wr3[:], 0.0)
            for k in range(4):
                nc.sync.dma_start(wr1[32 * k:32 * k + 32,
                                      32 * k:32 * k + 32],
                                  wF[256:288, 0:32])
                nc.sync.dma_start(wr2[32 * k:32 * k + 32,
                                      8 * k:8 * k + 8],
                                  wF[288:320, 0:8])
                nc.sync.dma_start(wr3[8 * k:8 * k + 8, 8 * k:8 * k + 8],
                                  wF[320:328, 0:8])
            with tc.For_i(0, NB4, 1) as i:
                cs = bass.ts(i, 512)
                for l in range(5):
                    src = xr if l == 0 else rh
                    ps = psm.tile([128, 512], dt.float32)
                    nc.tensor.matmul(ps[:], wr1[:], src[:, cs],
                                     start=True, stop=False)
                    nc.tensor.matmul(ps[:], ident[:], xr[:, cs],
                                     start=False, stop=True)
                    nc.scalar.activation(rh[:, cs], ps[:], AF.Prelu,
                                         bias=bx[:, 4:5], scale=1.0,
                                         alpha=0.25)
                ps = psm.tile([128, 512], dt.float32)
                nc.tensor.matmul(ps[0:32, :], wr2[:], rh[:, cs],
                                 start=True, stop=True)
                nc.scalar.activation(rf2[0:32, cs], ps[0:32, :], AF.Prelu,
                                     bias=bx[0:32, 5:6], scale=1.0,
                                     alpha=0.25)
                first = True
                for l in range(5):
                    src = rf2 if first else rh
                    first = False
                    ps = psm.tile([128, 512], dt.float32)
                    nc.tensor.matmul(ps[0:32, :], wr3[:], src[0:32, cs],
                                     start=True, stop=False)
                    nc.tensor.matmul(ps[0:32, :], ident[0:32, 0:32],
                                     rf2[0:32, cs], start=False, stop=True)
                    nc.scalar.activation(rh[0:32, cs], ps[0:32, :], AF.Prelu,
                                         bias=bx[0:32, 6:7], scale=1.0,
                                         alpha=0.25)
                bst = ap_.tile([128, 128], dt.float32)
                for k in range(4):
                    rstg = ap_.tile([32, 128], dt.bfloat16, tag="rstg")
                    nc.vector.tensor_copy(
                        rstg[:], rh[0:32, bass.ds(i * 512 + 128 * k, 128)])
                    tp = pst.tile([128, 128], dt.bfloat16)
                    nc.tensor.transpose(tp[:, 0:32], rstg[:],
                                        ident[0:32, 0:32])
                    nc.vector.tensor_copy(bst[:, 32 * k:32 * k + 32],
                                          tp[:, 0:32])
                nc.sync.dma_start(be_d[:, bass.ts(i, 128)], bst[:])

    # ---------------- C4/C5: message passing ----------------------------
    if PH < 4:
        return _finish_stub(nc, tile, mybir, o3_d, VSH)
    for it in range(ITER):
        with tile.TileContext(nc) as tc:
            with (
                tc.tile_pool(name=f"m{it}r", bufs=1) as res,
                tc.tile_pool(name=f"m{it}s", bufs=8) as sp,
                tc.tile_pool(name=f"m{it}p", bufs=2, space="PSUM") as psh,
            ):
                A_sb = res.tile([128, C2P], dt.bfloat16)
                b_sb = res.tile([128, C4P // 4], dt.float32)
                li8 = res.tile([128, NBLK4], dt.uint8)
                li32 = res.tile([128, NBLK4], dt.int32)
                ixe16 = res.tile([128, NBLK4], dt.uint16)
                ixe32 = res.tile([128, NBLK4], dt.int32)
                iota_oh = res.tile([128, 128], dt.int32)
                nc.sync.dma_start(A_sb[:], ae_d[:, :])
                nc.sync.dma_start(b_sb[:], be_d[:, :])
                nc.sync.dma_start(li8[:], li_d[:, :])
                nc.sync.dma_start(ixe16[:], ixe_d[:, :])
                nc.vector.tensor_copy(li32[:], li8[:])
                nc.vector.tensor_copy(ixe32[:], ixe16[:])
                nc.gpsimd.iota(iota_oh[:], pattern=[[1, 128]], base=0,
                               channel_multiplier=0)
                htab = h0F if it == 0 else H1F
                if it == 1:
                    o0f = res.tile([32, VSH], dt.float32)
                    o0h = res.tile([8, VSH], dt.float32)
                    ident = res.tile([128, 128], dt.bfloat16)
                    make_identity(nc, ident[:])
                with tc.For_i(0, NWC, 1) as w:
                    hp = psh.tile([128, 8], dt.float32)
                    for j in range(BPW):
                        st = sp.tile([128, 1], dt.int32)
                        nc.vector.tensor_copy(
                            st[:], ixe32[:, bass.ds(w * BPW + j, 1)])
                        he = sp.tile([128, 8], dt.bfloat16)
                        nc.gpsimd.indirect_dma_start(
                            out=he[:], out_offset=None, in_=htab[:, :],
                            in_offset=bass.IndirectOffsetOnAxis(
                                ap=st[:, 0:1], axis=0))
                        prod = sp.tile([128, 64], dt.float32)
                        nc.vector.tensor_tensor(
                            out=prod[:].rearrange("p (u j) -> p u j", u=8),
                            in0=A_sb[:, bass.ds(w * (BPW * 64) + 64 * j, 64)]
                                .rearrange("p (u j) -> p u j", u=8),
                            in1=he[:].rearrange("p (o j) -> p o j", o=1)
                                .to_broadcast([128, 8, 8]),
                            op=mybir.AluOpType.mult)
                        msum = sp.tile([128, 8], dt.float32)
                        nc.vector.tensor_reduce(
                            out=msum[:],
                            in_=prod[:].rearrange("p (u j) -> p u j", u=8),
                            axis=mybir.AxisListType.X,
                            op=mybir.AluOpType.add)
                        msgb = sp.tile([128, 8], dt.bfloat16)
                        nc.vector.tensor_tensor(
                            out=msgb[:], in0=msum[:],
                            in1=b_sb[:, bass.ds(w * (BPW * 8) + 8 * j, 8)],
                            op=mybir.AluOpType.add)
                        oh = sp.tile([128, 128], dt.bfloat16)
                        nc.vector.tensor_tensor(
                            out=oh[:],
                            in0=li32[:, bass.ds(w * BPW + j, 1)]
                                .to_broadcast([128, 128]),
                            in1=iota_oh[:],
                            op=mybir.AluOpType.is_equal)
                        nc.tensor.matmul(hp[:], oh[:], msgb[:],
                                         start=(j == 0), stop=(j == BPW - 1))
                    if it == 0:
                        hw1 = sp.tile([128, 8], dt.bfloat16)
                        nc.scalar.activation(hw1[:], hp[:], AF.Copy,
                                             scale=SCALE)
                        nc.sync.dma_start(H1d[bass.ds(w * 128, 128), :],
                                          hw1[:])
                    else:
                        # o0 rows 32:40 <- H2 (feature-major)
                        hw1f = sp.tile([128, 8], dt.bfloat16)
                        nc.scalar.activation(hw1f[:], hp[:], AF.Copy)
                        tph = psh.tile([128, 128], dt.bfloat16, tag="tph")
                        nc.tensor.transpose(tph[0:8, :], hw1f[:], ident[:])
                        nc.scalar.activation(o0h[:, bass.ts(w, 128)],
                                             tph[0:8, :], AF.Copy)
                        # o0 rows 0:32 <- node features of this core's shard
                        ftw = sp.tile([128, 32], dt.bfloat16, tag="ftw")
                        nc.sync.dma_start(ftw[:],
                                          ft_d[bass.ds(w * 128, 128), :])
                        tpf = psh.tile([128, 128], dt.bfloat16, tag="tpf")
                        nc.tensor.transpose(tpf[0:32, :], ftw[:], ident[:])
                        nc.scalar.activation(o0f[:, bass.ts(w, 128)],
                                             tpf[0:32, :], AF.Copy)
                if it == 0:
                    nc.gpsimd.collective_compute(
                        "AllGather", OP.bypass, replica_groups=grp,
                        ins=[H1d[:, :].opt()], outs=[H1F[:, :].opt()])
                else:
                    nc.sync.dma_start(o0_d[0:32, :], o0f[:])
                    nc.sync.dma_start(o0_d[32:40, :], o0h[:])

    # ---------------- R1-R10: readout BN iterations ---------------------
    if PH < 6:
        return _finish_stub(nc, tile, mybir, o3_d, VSH)
    CHK = 448
    NCHK = VSH // CHK
    assert NCHK * CHK == VSH
    NPAD = VP - V   # pad columns, all on the last core
    for r in range(10):
        with tile.TileContext(nc) as tc:
            with (
                tc.tile_pool(name=f"r{r}", bufs=1) as res,
                tc.tile_pool(name=f"r{r}s", bufs=3) as sp,
                tc.tile_pool(name=f"r{r}p", bufs=2, space="PSUM") as psp,
                tc.tile_pool(name=f"r{r}q", bufs=2, space="PSUM") as psq,
            ):
                o = res.tile([40, VSH], dt.float32)
                o0t = res.tile([40, VSH], dt.float32)
                sqt = res.tile([40, VSH], dt.float32)
                wro = res.tile([40, 96], dt.float32)
                brd = res.tile([40, 8], dt.float32)
                i40 = res.tile([40, 40], dt.float32)
                opad = res.tile([40, 1], dt.float32)
                ident = res.tile([128, 128], dt.bfloat16)
                nc.sync.dma_start(o[:], o0_d[:, :] if r == 0 else od_d[:, :])
                nc.sync.dma_start(o0t[:], o0_d[:, :])
                nc.sync.dma_start(wro[:], wro_d[:, :])
                nc.sync.dma_start(brd[:], brd_d[:, :])
                make_identity(nc, ident[:])
                nc.vector.tensor_copy(i40[:], ident[0:40, 0:40])
                if r == 0:
                    nc.vector.memset(opad[:], 0.0)
                else:
                    nc.sync.dma_start(opad[:], opad_d[:, :])
                with tc.For_i(0, NCHK, 1) as i:
                    cs = bass.ts(i, CHK)
                    ps = psp.tile([40, CHK], dt.float32)
                    nc.tensor.matmul(ps[:], wro[0:40, 0:40], o[:, cs],
                                     start=True, stop=True)
                    t1 = sp.tile([40, CHK], dt.float32)
                    nc.scalar.activation(t1[:], ps[:], AF.Prelu,
                                         bias=brd[0:40, 0:1], scale=1.0,
                                         alpha=0.25)
                    ps2 = psp.tile([40, CHK], dt.float32)
                    nc.tensor.matmul(ps2[:], wro[0:40, 40:80], t1[:],
                                     start=True, stop=False)
                    nc.tensor.matmul(ps2[:], i40[:], o0t[:, cs],
                                     start=False, stop=True)
                    nc.scalar.activation(o[:, cs], ps2[:], AF.Prelu,
                                         bias=brd[0:40, 1:2], scale=1.0,
                                         alpha=0.25)
                # pad-column chain (zero input, no residual)
                psa = psq.tile([40, 1], dt.float32)
                nc.tensor.matmul(psa[:], wro[0:40, 0:40], opad[:],
                                 start=True, stop=True)
                t1p = sp.tile([40, 1], dt.float32, tag="t1p")
                nc.scalar.activation(t1p[:], psa[:], AF.Prelu,
                                     bias=brd[0:40, 0:1], scale=1.0,
                                     alpha=0.25)
                psb = psq.tile([40, 1], dt.float32)
                nc.tensor.matmul(psb[:], wro[0:40, 40:80], t1p[:],
                                 start=True, stop=True)
                nc.scalar.activation(opad[:], psb[:], AF.Prelu,
                                     bias=brd[0:40, 1:2], scale=1.0,
                                     alpha=0.25)
                # stats + AllReduce
                arin = res.tile([40, 2], dt.float32)
                nc.vector.tensor_reduce(out=arin[:, 0:1], in_=o[:],
                                        axis=mybir.AxisListType.X,
                                        op=OP.add)
                nc.scalar.activation(sqt[:], o[:], AF.Square)
                nc.vector.tensor_reduce(out=arin[:, 1:2], in_=sqt[:],
                                        axis=mybir.AxisListType.X,
                                        op=OP.add)
                ars = res.tile([40, 2], dt.float32)
                if os.environ.get("K2_NOAR"):
                    nc.vector.tensor_copy(ars[:], arin[:])
                else:
                    nc.sync.dma_start(arb_d[:, :], arin[:])
                    nc.gpsimd.collective_compute(
                        "AllReduce", OP.add, replica_groups=grp,
                        ins=[arb_d[:, :].opt()], outs=[arB_d[:, :].opt()])
                    nc.sync.dma_start(ars[:], arB_d[:, :])
                # corrections for NPAD all-zero-input pad columns
                tca = res.tile([40, 8], dt.float32)
                nc.scalar.activation(tca[:, 0:1], opad[:], AF.Copy,
                                     scale=float(NPAD))
                nc.vector.tensor_tensor(out=tca[:, 1:2], in0=ars[:, 0:1],
                                        in1=tca[:, 0:1], op=OP.subtract)
                nc.scalar.activation(tca[:, 2:3], opad[:], AF.Square)
                nc.scalar.activation(tca[:, 3:4], tca[:, 2:3], AF.Copy,
                                     scale=float(NPAD))
                nc.vector.tensor_tensor(out=tca[:, 4:5], in0=ars[:, 1:2],
                                        in1=tca[:, 3:4], op=OP.subtract)
                # mean / var / rstd
                mean = res.tile([40, 4], dt.float32)
                nc.scalar.activation(mean[:, 0:1], tca[:, 1:2], AF.Copy,
                                     scale=1.0 / V)
                nc.scalar.activation(mean[:, 1:2], tca[:, 4:5], AF.Copy,
                                     scale=1.0 / V)
                nc.scalar.activation(mean[:, 2:3], mean[:, 0:1], AF.Square)
                nc.vector.tensor_tensor(out=mean[:, 3:4], in0=mean[:, 1:2],
                                        in1=mean[:, 2:3], op=OP.subtract)
                veps = res.tile([40, 4], dt.float32)
                nc.vector.tensor_scalar_add(veps[:, 0:1], mean[:, 3:4], EPS)
                nc.vector.reciprocal(veps[:, 1:2], veps[:, 0:1])
                nc.scalar.sqrt(veps[:, 2:3], veps[:, 1:2])
                sc = res.tile([40, 4], dt.float32)
                nc.vector.tensor_tensor(out=sc[:, 0:1], in0=veps[:, 2:3],
                                        in1=brd[0:40, 2:3], op=OP.mult)
                nc.vector.tensor_tensor(out=sc[:, 1:2], in0=mean[:, 0:1],
                                        in1=sc[:, 0:1], op=OP.mult)
                nc.vector.tensor_tensor(out=sc[:, 2:3], in0=brd[0:40, 3:4],
                                        in1=sc[:, 1:2], op=OP.subtract)
                nc.scalar.activation(o[:], o[:], AF.Identity,
                                     bias=sc[:, 2:3], scale=sc[:, 0:1])
                nc.scalar.activation(opad[:], opad[:], AF.Identity,
                                     bias=sc[:, 2:3], scale=sc[:, 0:1])
                nc.sync.dma_start(od_d[:, :], o[:])
                nc.sync.dma_start(opad_d[:, :], opad[:])

    # ---------------- R11: l2 head + l3 tail -> o3 ----------------------
    with tile.TileContext(nc) as tc:
        with (
            tc.tile_pool(name="r11", bufs=1) as res,
            tc.tile_pool(name="r11s", bufs=3) as sp,
            tc.tile_pool(name="r11p", bufs=2, space="PSUM") as psp,
        ):
            o = res.tile([40, VSH], dt.float32)
            o3sb = res.tile([2, VSH], dt.bfloat16)
            wro = res.tile([40, 96], dt.float32)
            brd = res.tile([40, 8], dt.float32)
            i40 = res.tile([40, 40], dt.float32)
            ident = res.tile([128, 128], dt.bfloat16)
            nc.sync.dma_start(o[:], od_d[:, :])
            nc.sync.dma_start(wro[:], wro_d[:, :])
            nc.sync.dma_start(brd[:], brd_d[:, :])
            make_identity(nc, ident[:])
            nc.vector.tensor_copy(i40[:], ident[0:40, 0:40])
            with tc.For_i(0, NCHK, 1) as i:
                cs = bass.ts(i, CHK)
                ps = psp.tile([40, CHK], dt.float32)
                nc.tensor.matmul(ps[0:2, :], wro[0:40, 80:82], o[:, cs],
                                 start=True, stop=True)
                o2c = sp.tile([2, CHK], dt.float32, tag="o2c")
                nc.scalar.activation(o2c[:], ps[0:2, :], AF.Prelu,
                                     bias=brd[0:2, 4:5], scale=1.0,
                                     alpha=0.25)
                o3c = o2c
                for l in range(10):
                    psl = psp.tile([40, CHK], dt.float32)
                    nc.tensor.matmul(psl[0:2, :], wro[0:2, 82:84], o3c[:],
                                     start=True, stop=True)
                    t3 = sp.tile([2, CHK], dt.float32, tag="t3")
                    nc.scalar.activation(t3[:], psl[0:2, :], AF.Prelu,
                                         bias=brd[0:2, 5:6], scale=1.0,
                                         alpha=0.25)
                    psl2 = psp.tile([40, CHK], dt.float32)
                    nc.tensor.matmul(psl2[0:2, :], wro[0:2, 84:86], t3[:],
                                     start=True, stop=False)
                    nc.tensor.matmul(psl2[0:2, :], i40[0:2, 0:2], o2c[:],
                                     start=False, stop=True)
                    o3n = sp.tile([2, CHK], dt.float32, tag="o3n")
                    nc.scalar.activation(o3n[:], psl2[0:2, :], AF.Prelu,
                                         bias=brd[0:2, 6:7], scale=1.0,
                                         alpha=0.25)
                    o3c = o3n
                nc.vector.tensor_copy(o3sb[:, cs], o3c[:])
            nc.sync.dma_start(o3_d[:, :], o3sb[:])

    nc.compile()
    return nc


def _finish_stub(nc, tile, mybir, o3_d, VSH):
    dt = mybir.dt
    with tile.TileContext(nc) as tc:
        with tc.tile_pool(name="stub", bufs=1) as p:
            t = p.tile([2, VSH], dt.bfloat16)
            nc.vector.memset(t[:], 0.0)
            nc.sync.dma_start(o3_d[:, :], t[:])
    nc.compile()
    return nc


def _prelu(x, a):
    return np.where(x >= 0, x, a * x)


_PACK_CACHE = {}


def _host_pack(X_Node, X_Neis):
    import hashlib
    hkey = hashlib.blake2b(X_Node.tobytes(), digest_size=16)
    hkey.update(X_Neis.tobytes())
    hkey = hkey.digest()
    hit = _PACK_CACHE.get(hkey)
    if hit is not None:
        return hit
    xn = X_Node.astype(np.int32)
    win = xn >> 7
    counts = np.bincount(win, minlength=NW)
    BPW = max(9, int(-(-int(counts.max()) // 128)))
    NBLK4 = -(-(NWC * BPW) // 4) * 4
    order = np.argsort(win, kind="stable").astype(np.int32)
    starts = np.zeros(NW + 1, np.int32)
    np.cumsum(counts, out=starts[1:])
    ws = win[order]
    r = np.arange(E, dtype=np.int32) - starts[ws]
    core = ws // NWC
    bcol = (ws % NWC) * BPW + (r >> 7)
    prow = r & 127
    li8 = np.full((NCORES, 128, NBLK4), 128, np.uint8)
    ixe16 = np.zeros((NCORES, 128, NBLK4), np.uint16)
    li8[core, prow, bcol] = (xn[order] & 127).astype(np.uint8)
    ixe16[core, prow, bcol] = X_Neis[order].astype(np.uint16)
    _PACK_CACHE[hkey] = (BPW, li8, ixe16)
    return BPW, li8, ixe16


def kernel(**inputs):
    X_Node = np.asarray(inputs["X_Node"]).astype(np.int64)
    X_Neis = np.asarray(inputs["X_Neis"]).astype(np.int64)
    fM = np.asarray(inputs["feature_Matrix"], dtype=np.float32)
    H0 = np.asarray(inputs["node_states"], dtype=np.float32)
    g = {k: np.asarray(v, dtype=np.float32) for k, v in inputs.items()
         if k not in ("X_Node", "X_Neis")}

    BPW, li8, ixe16 = _host_pack(X_Node, X_Neis)

    ftab = np.zeros((VP, 32), BF16)
    ftab[:V] = fM.T.astype(BF16)
    h0tab = np.zeros((VP, 8), BF16)
    h0tab[:V] = (H0 * SCALE).astype(BF16)

    wfull = np.zeros((WR, 64), BF16)
    for k, wname in enumerate(["xi1w", "xi2w", "xi3w", "xi3aw"]):
        wfull[64 * k:64 * k + 64] = g[wname].T.astype(BF16)
    wfull[256:288, 0:32] = g["r1w"].T.astype(BF16)
    wfull[288:320, 0:8] = g["r2w"].T.astype(BF16)
    wfull[320:328, 0:8] = g["r3aw"].T.astype(BF16)

    bxv = np.zeros((128, 8), np.float32)
    for i, bn in enumerate(["xi1b", "xi2b", "xi3b", "xi3ab"]):
        bxv[0:64, i] = g[bn]
        bxv[64:128, i] = g[bn]
    bxv[:, 4] = np.tile(g["r1b"], 4)
    bxv[0:32, 5] = np.tile(g["r2b"], 4)
    bxv[0:32, 6] = np.tile(g["r3ab"], 4)

    wro = np.zeros((40, 96), np.float32)
    wro[0:40, 0:40] = g["l1w"].T
    wro[0:40, 40:80] = g["l1aw"].T
    wro[0:40, 80:82] = g["l2w"].T
    wro[0:2, 82:84] = g["l3w"].T
    wro[0:2, 84:86] = g["l3aw"].T
    brd = np.zeros((40, 8), np.float32)
    brd[0:40, 0] = g["l1b"]
    brd[0:40, 1] = g["l1ab"]
    brd[0:40, 2] = g["bn_g"]
    brd[0:40, 3] = g["bn_b"]
    brd[0:2, 4] = g["l2b"]
    brd[0:2, 5] = g["l3b"]
    brd[0:2, 6] = g["l3ab"]

    in_maps = []
    for c in range(NCORES):
        in_maps.append({
            "ftsh": ftab[VSH * c:VSH * (c + 1)],
            "h0sh": h0tab[VSH * c:VSH * (c + 1)],
            "wsh": wfull[WSH * c:WSH * (c + 1)],
            "li8": li8[c], "ixe": ixe16[c], "bx": bxv,
            "wro": wro, "brd": brd,
        })

    PH = int(os.environ.get("K2_PHASES", "99"))
    key = ("nc", BPW, PH, os.environ.get("K2_NOAR", ""))
    if key not in _CACHE:
        _CACHE[key] = _build_nc(BPW, PH)
    nc = _CACHE[key]

    t0 = _time.time()
    res = bass_utils.run_bass_kernel_spmd(
        nc, in_maps, core_ids=list(range(NCORES)), trace=False)
    LAST_RESULT["run_wall_s"] = _time.time() - t0
    LAST_RESULT["exec_time_ns"] = res.exec_time_ns

    o3f = np.concatenate([res.results[c]["o3f"] for c in range(NCORES)],
                         axis=1)[:, :V]
    return np.concatenate([o3f[0], o3f[1]], axis=0).astype(np.float32)


# revision 6
# speedup vs baseline: 1.1450x; 1.1450x over previous
"""Trainium2 kernel for nn_Non_LinearGNN: 8-core SPMD, For_i hardware loops.

Sharding: each core owns 49 contiguous windows of 128 nodes; edges are
sorted by X_Node and packed into BPW blocks of 128 per window (~89% fill).
The per-window segment-sum is exact locally (no AllReduce); one AllGather
publishes H1 for the iter-2 gather, and the final H2 shard is the output.
Node-feature gathers use the core's own aligned shard with local indices
(li + 128*wl); neighbor gathers use the AllGathered global tables.

Program structure: TileContexts with one For_i hardware loop each
(multiple For_i in one context miscompile on this HW path; contexts are
separated by the TileContext drain/semaphore-clear barrier):
  C0  static setup + AllGathers of the feature table / H0 table / weights
  C1  gather loop: per block-pair, 4 indirect DMAs + transpose -> X 2-pack
  C2  Xi MLP loop: 16 fused 64-d layers (block-diag 128x128 weights) -> A
  C3  Rou MLP loop: 11 fused 32/8-d layers (block-diag 4-pack) -> b
  C4  message-passing iter 0 (one-hot scatter matmul) + H1 AllGather
  C5  message-passing iter 1 -> H2 + builds o0=[featT|H2] feature-major
  R1-10 readout BN iterations: chunked f32 MLP + residual, global batch
      stats via a [40,2] AllReduce with exact pad-column correction
  R11 l2 head + 10 l3 tail layers -> o3 [2, VSH] output shard
Host: vectorized (and hash-cached) edge binning only; the readout MLP
runs on device. Why hardware loops: the dominant per-call cost on the
axon-tunneled runtime is shipping/loading the program (~15us per BIR
instruction); For_i bodies are stored once, cutting the NEFF ~10x.
"""

import os
import sys
import time as _time

import numpy as np

sys.path.insert(0, "/opt/trn_rl_repo")

import ml_dtypes

try:
    import jax
    jax.config.update("jax_compilation_cache_dir", "/tmp/jaxcache")
    jax.config.update("jax_persistent_cache_min_entry_size_bytes", -1)
    jax.config.update("jax_persistent_cache_min_compile_time_secs", 0)
except Exception:
    pass

import concourse.bacc as bacc
import concourse.mybir as mybir
import concourse.tile as tile
from concourse import bass, bass_utils
from concourse.masks import make_identity

BF16 = ml_dtypes.bfloat16

V = 50000
E = 400000
LN = 32
S = 8
ITER = 2
DEG = 8.0
MU = 0.8
D = LN + S
EPS = 1e-5
SCALE = MU / S / DEG
NCORES = 8

NW = 392                  # node windows of 128 (covers VP)
NWC = NW // NCORES        # 49 windows per core
VP = NW * 128             # 50176
VSH = VP // NCORES        # 6272 rows per core shard
WR = 336                  # weight-table rows (padded)
WSH = WR // NCORES        # 42

_CACHE = {}
LAST_RESULT = {}


def _build_nc(BPW, PH=99):
    NBLK4 = -(-(NWC * BPW) // 4) * 4        # blocks padded to mult of 4
    NPAIR = NBLK4 // 2
    NQ4 = NBLK4 // 4
    C2 = NBLK4 * 64                          # 2-pack cols
    C2P = -(-C2 // 512) * 512                # padded to 512
    NB2 = C2P // 512
    C4 = NBLK4 * 32                          # 4-pack cols
    C4P = -(-C4 // 512) * 512
    NB4 = C4P // 512
    assert C2P - C2 >= 128 or C2P == C2

    nc = bacc.Bacc("TRN2", target_bir_lowering=False, debug=False)
    dt = mybir.dt
    AF = mybir.ActivationFunctionType
    OP = mybir.AluOpType
    grp = [list(range(NCORES))]

    ft_d = nc.declare_dram_parameter("ftsh", [VSH, 32], dt.bfloat16,
                                     isOutput=False)
    h0_d = nc.declare_dram_parameter("h0sh", [VSH, 8], dt.bfloat16,
                                     isOutput=False)
    w_d = nc.declare_dram_parameter("wsh", [WSH, 64], dt.bfloat16,
                                    isOutput=False)
    li_d = nc.declare_dram_parameter("li8", [128, NBLK4], dt.uint8,
                                     isOutput=False)
    ixe_d = nc.declare_dram_parameter("ixe", [128, NBLK4], dt.uint16,
                                      isOutput=False)
    bx_d = nc.declare_dram_parameter("bx", [128, 8], dt.float32,
                                     isOutput=False)
    wro_d = nc.declare_dram_parameter("wro", [40, 96], dt.float32,
                                      isOutput=False)
    brd_d = nc.declare_dram_parameter("brd", [40, 8], dt.float32,
                                      isOutput=False)
    o3_d = nc.declare_dram_parameter("o3f", [2, VSH], dt.bfloat16,
                                     isOutput=True)

    ftF = nc.dram_tensor("ftF", [VP, 32], dt.bfloat16, kind="Internal",
                         addr_space="Shared")
    h0F = nc.dram_tensor("h0F", [VP, 8], dt.bfloat16, kind="Internal",
                         addr_space="Shared")
    wF = nc.dram_tensor("wF", [WR, 64], dt.bfloat16, kind="Internal",
                        addr_space="Shared")
    H1F = nc.dram_tensor("H1F", [VP, 8], dt.bfloat16, kind="Internal",
                         addr_space="Shared")
    ftL = nc.dram_tensor("ftL", [VSH + 128, 32], dt.bfloat16, kind="Internal")
    ft_b = nc.dram_tensor("ft_b", [VSH, 32], dt.bfloat16, kind="Internal")
    h0_b = nc.dram_tensor("h0_b", [VSH, 8], dt.bfloat16, kind="Internal")
    w_b = nc.dram_tensor("w_b", [WSH, 64], dt.bfloat16, kind="Internal")
    H1d = nc.dram_tensor("H1d", [VSH, 8], dt.bfloat16, kind="Internal")
    xp_d = nc.dram_tensor("xp_d", [128, C2P], dt.bfloat16, kind="Internal")
    xr_d = nc.dram_tensor("xr_d", [128, C4P], dt.bfloat16, kind="Internal")
    ae_d = nc.dram_tensor("ae_d", [128, C2P], dt.bfloat16, kind="Internal")
    be_d = nc.dram_tensor("be_d", [128, C4P // 4], dt.float32, kind="Internal")
    he_d = nc.dram_tensor("he_d", [128, NBLK4 * 8], dt.bfloat16,
                          kind="Internal")
    o0_d = nc.dram_tensor("o0_d", [40, VSH], dt.float32, kind="Internal")
    od_d = nc.dram_tensor("od_d", [40, VSH], dt.float32, kind="Internal")
    opad_d = nc.dram_tensor("opad_d", [40, 1], dt.float32, kind="Internal")
    arb_d = nc.dram_tensor("arb_d", [40, 2], dt.float32, kind="Internal")
    arB_d = nc.dram_tensor("arB_d", [40, 2], dt.float32, kind="Internal",
                           addr_space="Shared")

    # ---------------- C0: stage + AllGather tables ----------------
    with tile.TileContext(nc) as tc:
        with tc.tile_pool(name="c0", bufs=1) as p0:
            zt = p0.tile([128, 32], dt.bfloat16)
            nc.vector.memset(zt[:], 0.0)
            nc.sync.dma_start(ft_b[:, :], ft_d[:, :])
            nc.sync.dma_start(ftL[0:VSH, :], ft_d[:, :])
            nc.sync.dma_start(ftL[VSH:VSH + 128, :], zt[:])
            nc.sync.dma_start(h0_b[:, :], h0_d[:, :])
            nc.sync.dma_start(w_b[:, :], w_d[:, :])
            nc.gpsimd.collective_compute(
                "AllGather", OP.bypass, replica_groups=grp,
                ins=[ft_b[:, :].opt()], outs=[ftF[:, :].opt()])
            nc.gpsimd.collective_compute(
                "AllGather", OP.bypass, replica_groups=grp,
                ins=[h0_b[:, :].opt()], outs=[h0F[:, :].opt()])
            nc.gpsimd.collective_compute(
                "AllGather", OP.bypass, replica_groups=grp,
                ins=[w_b[:, :].opt()], outs=[wF[:, :].opt()])

    # ---------------- C1: gather loop -> xp (2-pack X), xr (4-pack) -----
    if PH < 1:
        return _finish_stub(nc, tile, mybir, o3_d, VSH)
    with tile.TileContext(nc) as tc:
        with (
            tc.tile_pool(name="c1r", bufs=1) as res,
            tc.tile_pool(name="c1g", bufs=4) as gp,
            tc.tile_pool(name="c1s", bufs=8) as sp,
            tc.tile_pool(name="c1p", bufs=2, space="PSUM") as pst,
        ):
            li8 = res.tile([128, NBLK4], dt.uint8)
            ixe16 = res.tile([128, NBLK4], dt.uint16)
            ixe32 = res.tile([128, NBLK4], dt.int32)
            ixn32 = res.tile([128, NBLK4], dt.int32)
            wb = res.tile([128, NBLK4], dt.int32)
            ident = res.tile([128, 128], dt.bfloat16)
            xp = res.tile([128, C2P], dt.bfloat16)
            nc.sync.dma_start(li8[:], li_d[:, :])
            nc.sync.dma_start(ixe16[:], ixe_d[:, :])
            nc.vector.tensor_copy(ixe32[:], ixe16[:])
            make_identity(nc, ident[:])
            nc.vector.memset(xp[:], 0.0)
            # local window base per block: 128 * (b // BPW), b < NWC*BPW
            nc.gpsimd.iota(wb[:, 0:NWC * BPW],
                           pattern=[[128, NWC], [0, BPW]],
                           base=0, channel_multiplier=0)
            if NBLK4 > NWC * BPW:
                nc.vector.memset(wb[:, NWC * BPW:], 0.0)
            nc.vector.tensor_copy(ixn32[:], li8[:])
            nc.vector.tensor_tensor(out=ixn32[:], in0=ixn32[:], in1=wb[:],
                                    op=OP.add)
            with tc.For_i(0, NPAIR, 1) as q:
                g = gp.tile([128, 128], dt.bfloat16)
                for k in range(2):
                    stn = sp.tile([128, 1], dt.int32)
                    nc.vector.tensor_copy(stn[:],
                                          ixn32[:, bass.ds(q * 2 + k, 1)])
                    nc.gpsimd.indirect_dma_start(
                        out=g[:, 64 * k:64 * k + 32], out_offset=None,
                        in_=ftL[:, :],
                        in_offset=bass.IndirectOffsetOnAxis(
                            ap=stn[:, 0:1], axis=0))
                    ste = sp.tile([128, 1], dt.int32)
                    nc.vector.tensor_copy(ste[:],
                                          ixe32[:, bass.ds(q * 2 + k, 1)])
                    nc.gpsimd.indirect_dma_start(
                        out=g[:, 64 * k + 32:64 * k + 64], out_offset=None,
                        in_=ftF[:, :],
                        in_offset=bass.IndirectOffsetOnAxis(
                            ap=ste[:, 0:1], axis=0))
                tp = pst.tile([128, 128], dt.bfloat16)
                nc.tensor.transpose(tp[:], g[:], ident[:])
                nc.scalar.activation(xp[:, bass.ts(q, 128)], tp[:], AF.Copy)
            # xr 4-pack: lane a of quad m holds block 4m+a
            xr = res.tile([128, C4P], dt.bfloat16)
            nc.vector.memset(xr[:], 0.0)
            for a in range(4):
                rs = 64 * (a % 2)
                co = 128 * (a // 2)
                nc.vector.tensor_copy(
                    xr[32 * a:32 * a + 32, 0:NQ4 * 128].rearrange(
                        "p (m c) -> p m c", c=128),
                    xp[rs:rs + 32, co:co + NQ4 * 256].rearrange(
                        "p (m c) -> p m c", c=256)[:, :, 0:128])
            nc.sync.dma_start(xp_d[:, :], xp[:])
            nc.sync.dma_start(xr_d[:, :], xr[:])

    # ---------------- C2: Xi MLP loop -> A edge-major -------------------
    if PH < 2:
        return _finish_stub(nc, tile, mybir, o3_d, VSH)
    with tile.TileContext(nc) as tc:
        with (
            tc.tile_pool(name="c2r", bufs=1) as res,
            tc.tile_pool(name="c2a", bufs=2) as ap_,
            tc.tile_pool(name="c2m", bufs=2, space="PSUM") as psm,
            tc.tile_pool(name="c2t", bufs=2, space="PSUM") as pst,
        ):
            xp = res.tile([128, C2P], dt.bfloat16)
            ht = res.tile([128, C2P], dt.bfloat16)
            f2t = res.tile([128, C2P], dt.bfloat16)
            wxi = res.tile([128, 512], dt.bfloat16)
            ident = res.tile([128, 128], dt.bfloat16)
            bx = res.tile([128, 8], dt.float32)
            nc.sync.dma_start(xp[:], xp_d[:, :])
            nc.sync.dma_start(bx[:], bx_d[:, :])
            make_identity(nc, ident[:])
            nc.vector.memset(wxi[:], 0.0)
            for k in range(4):
                nc.sync.dma_start(wxi[0:64, 128 * k:128 * k + 64],
                                  wF[64 * k:64 * k + 64, :])
                nc.sync.dma_start(wxi[64:128, 128 * k + 64:128 * k + 128],
                                  wF[64 * k:64 * k + 64, :])
            with tc.For_i(0, NB2, 1) as i:
                cs = bass.ts(i, 512)
                for l in range(5):
                    src = xp if l == 0 else ht
                    ps = psm.tile([128, 512], dt.float32)
                    nc.tensor.matmul(ps[:], wxi[:, 0:128], src[:, cs],
                                     start=True, stop=False)
                    nc.tensor.matmul(ps[:], ident[:], xp[:, cs],
                                     start=False, stop=True)
                    nc.scalar.activation(ht[:, cs], ps[:], AF.Prelu,
                                         bias=bx[:, 0:1], scale=1.0,
                                         alpha=0.25)
                ps = psm.tile([128, 512], dt.float32)
                nc.tensor.matmul(ps[:], wxi[:, 128:256], ht[:, cs],
                                 start=True, stop=True)
                nc.scalar.activation(f2t[:, cs], ps[:], AF.Prelu,
                                     bias=bx[:, 1:2], scale=1.0, alpha=0.25)
                first = True
                for l in range(5):
                    ps = psm.tile([128, 512], dt.float32)
                    src = f2t if first else ht
                    first = False
                    nc.tensor.matmul(ps[:], wxi[:, 256:384], src[:, cs],
                                     start=True, stop=True)
                    nc.scalar.activation(ht[:, cs], ps[:], AF.Prelu,
                                         bias=bx[:, 2:3], scale=1.0,
                                         alpha=0.25)
                    ps = psm.tile([128, 512], dt.float32)
                    nc.tensor.matmul(ps[:], wxi[:, 384:512], ht[:, cs],
                                     start=True, stop=False)
                    nc.tensor.matmul(ps[:], ident[:], f2t[:, cs],
                                     start=False, stop=True)
                    nc.scalar.activation(ht[:, cs], ps[:], AF.Prelu,
                                         bias=bx[:, 3:4], scale=1.0,
                                         alpha=0.25)
                ast = ap_.tile([128, 512], dt.bfloat16)
                for k in range(4):
                    hstg = ap_.tile([128, 128], dt.bfloat16, tag="hstg")
                    nc.vector.tensor_copy(
                        hstg[:], ht[:, bass.ds(i * 512 + 128 * k, 128)])
                    tp = pst.tile([128, 128], dt.bfloat16)
                    nc.tensor.transpose(tp[:], hstg[:], ident[:])
                    nc.scalar.activation(ast[:, 128 * k:128 * k + 128],
                                         tp[:], AF.Copy)
                nc.sync.dma_start(ae_d[:, cs], ast[:])

    # ---------------- C3: Rou MLP loop -> b edge-major ------------------
    if PH < 3:
        return _finish_stub(nc, tile, mybir, o3_d, VSH)
    with tile.TileContext(nc) as tc:
        with (
            tc.tile_pool(name="c3r", bufs=1) as res,
            tc.tile_pool(name="c3a", bufs=2) as ap_,
            tc.tile_pool(name="c3m", bufs=2, space="PSUM") as psm,
            tc.tile_pool(name="c3t", bufs=2, space="PSUM") as pst,
        ):
            xr = res.tile([128, C4P], dt.bfloat16)
            rh = res.tile([128, C4P], dt.bfloat16)
            rf2 = res.tile([128, C4P], dt.bfloat16)
            wr1 = res.tile([128, 128], dt.bfloat16)
            wr2 = res.tile([128, 32], dt.bfloat16)
            wr3 = res.tile([32, 32], dt.bfloat16)
            ident = res.tile([128, 128], dt.bfloat16)
            bx = res.tile([128, 8], dt.float32)
            nc.sync.dma_start(xr[:], xr_d[:, :])
            nc.sync.dma_start(bx[:], bx_d[:, :])
            make_identity(nc, ident[:])
            nc.vector.memset(wr1[:], 0.0)
            nc.vector.memset(wr2[:], 0.0)
            nc.vector.memset(wr3[:], 0.0)
            for k in range(4):
                nc.sync.dma_start(wr1[32 * k:32 * k + 32,
                                      32 * k:32 * k + 32],
                                  wF[256:288, 0:32])
                nc.sync.dma_start(wr2[32 * k:32 * k + 32,
                                      8 * k:8 * k + 8],
                                  wF[288:320, 0:8])
                nc.sync.dma_start(wr3[8 * k:8 * k + 8, 8 * k:8 * k + 8],
                                  wF[320:328, 0:8])
            with tc.For_i(0, NB4, 1) as i:
                cs = bass.ts(i, 512)
                for l in range(5):
                    src = xr if l == 0 else rh
                    ps = psm.tile([128, 512], dt.float32)
                    nc.tensor.matmul(ps[:], wr1[:], src[:, cs],
                                     start=True, stop=False)
                    nc.tensor.matmul(ps[:], ident[:], xr[:, cs],
                                     start=False, stop=True)
                    nc.scalar.activation(rh[:, cs], ps[:], AF.Prelu,
                                         bias=bx[:, 4:5], scale=1.0,
                                         alpha=0.25)
                ps = psm.tile([128, 512], dt.float32)
                nc.tensor.matmul(ps[0:32, :], wr2[:], rh[:, cs],
                                 start=True, stop=True)
                nc.scalar.activation(rf2[0:32, cs], ps[0:32, :], AF.Prelu,
                                     bias=bx[0:32, 5:6], scale=1.0,
                                     alpha=0.25)
                first = True
                for l in range(5):
                    src = rf2 if first else rh
                    first = False
                    ps = psm.tile([128, 512], dt.float32)
                    nc.tensor.matmul(ps[0:32, :], wr3[:], src[0:32, cs],
                                     start=True, stop=False)
                    nc.tensor.matmul(ps[0:32, :], ident[0:32, 0:32],
                                     rf2[0:32, cs], start=False, stop=True)
                    nc.scalar.activation(rh[0:32, cs], ps[0:32, :], AF.Prelu,
                                         bias=bx[0:32, 6:7], scale=1.0,
                                         alpha=0.25)
                bst = ap_.tile([128, 128], dt.float32)
                for k in range(4):
                    rstg = ap_.tile([32, 128], dt.bfloat16, tag="rstg")
                    nc.vector.tensor_copy(
                        rstg[:], rh[0:32, bass.ds(i * 512 + 128 * k, 128)])
                    tp = pst.tile([128, 128], dt.bfloat16)
                    nc.tensor.transpose(tp[:, 0:32], rstg[:],
                                        ident[0:32, 0:32])
                    nc.vector.tensor_copy(bst[:, 32 * k:32 * k + 32],
                                          tp[:, 0:32])
                nc.sync.dma_start(be_d[:, bass.ts(i, 128)], bst[:])

    # ---------------- C4/C5: message passing ----------------------------
    if PH < 4:
        return _finish_stub(nc, tile, mybir, o3_d, VSH)
    for it in range(ITER):
        # gather loop: one indirect DMA per iteration (extra idmas in one
        # iteration serialize at ~20us each; single ones pipeline freely)
        with tile.TileContext(nc) as tc:
            with (
                tc.tile_pool(name=f"g{it}r", bufs=1) as res,
                tc.tile_pool(name=f"g{it}s", bufs=8) as sp,
            ):
                ixe16 = res.tile([128, NBLK4], dt.uint16)
                ixe32 = res.tile([128, NBLK4], dt.int32)
                he_all = res.tile([128, NBLK4 * 8], dt.bfloat16)
                nc.sync.dma_start(ixe16[:], ixe_d[:, :])
                nc.vector.tensor_copy(ixe32[:], ixe16[:])
                htab = h0F if it == 0 else H1F
                NBLKR = NWC * BPW
                with tc.For_i(0, NBLKR, 1) as b:
                    st = sp.tile([128, 1], dt.int32)
                    nc.vector.tensor_copy(st[:], ixe32[:, bass.ds(b, 1)])
                    he = sp.tile([128, 8], dt.bfloat16)
                    nc.gpsimd.indirect_dma_start(
                        out=he[:], out_offset=None, in_=htab[:, :],
                        in_offset=bass.IndirectOffsetOnAxis(
                            ap=st[:, 0:1], axis=0))
                    nc.vector.tensor_copy(he_all[:, bass.ts(b, 8)], he[:])
                nc.sync.dma_start(he_d[:, 0:NBLKR * 8],
                                  he_all[:, 0:NBLKR * 8])
        with tile.TileContext(nc) as tc:
            with (
                tc.tile_pool(name=f"m{it}r", bufs=1) as res,
                tc.tile_pool(name=f"m{it}s", bufs=8) as sp,
                tc.tile_pool(name=f"m{it}p", bufs=2, space="PSUM") as psh,
            ):
                A_sb = res.tile([128, C2P], dt.bfloat16)
                b_sb = res.tile([128, C4P // 4], dt.float32)
                li8 = res.tile([128, NBLK4], dt.uint8)
                li32 = res.tile([128, NBLK4], dt.int32)
                ixe16 = res.tile([128, NBLK4], dt.uint16)
                ixe32 = res.tile([128, NBLK4], dt.int32)
                iota_oh = res.tile([128, 128], dt.int32)
                nc.sync.dma_start(A_sb[:], ae_d[:, :])
                nc.sync.dma_start(b_sb[:], be_d[:, :])
                nc.sync.dma_start(li8[:], li_d[:, :])
                nc.sync.dma_start(ixe16[:], ixe_d[:, :])
                nc.vector.tensor_copy(li32[:], li8[:])
                nc.vector.tensor_copy(ixe32[:], ixe16[:])
                nc.gpsimd.iota(iota_oh[:], pattern=[[1, 128]], base=0,
                               channel_multiplier=0)
                he_sb = res.tile([128, NBLK4 * 8], dt.bfloat16)
                nc.sync.dma_start(he_sb[:], he_d[:, :])
                if it == 1:
                    o0f = res.tile([32, VSH], dt.float32)
                    o0h = res.tile([8, VSH], dt.float32)
                    ident = res.tile([128, 128], dt.bfloat16)
                    make_identity(nc, ident[:])
                with tc.For_i(0, NWC, 1) as w:
                    hp = psh.tile([128, 8], dt.float32)
                    for j in range(BPW):
                        he = sp.tile([128, 8], dt.bfloat16)
                        nc.vector.tensor_copy(
                            he[:], he_sb[:, bass.ds(w * (BPW * 8) + 8 * j, 8)])
                        prod = sp.tile([128, 64], dt.float32)
                        nc.vector.tensor_tensor(
                            out=prod[:].rearrange("p (u j) -> p u j", u=8),
                            in0=A_sb[:, bass.ds(w * (BPW * 64) + 64 * j, 64)]
                                .rearrange("p (u j) -> p u j", u=8),
                            in1=he[:].rearrange("p (o j) -> p o j", o=1)
                                .to_broadcast([128, 8, 8]),
                            op=mybir.AluOpType.mult)
                        msum = sp.tile([128, 8], dt.float32)
                        nc.vector.tensor_reduce(
                            out=msum[:],
                            in_=prod[:].rearrange("p (u j) -> p u j", u=8),
                            axis=mybir.AxisListType.X,
                            op=mybir.AluOpType.add)
                        msgb = sp.tile([128, 8], dt.bfloat16)
                        nc.vector.tensor_tensor(
                            out=msgb[:], in0=msum[:],
                            in1=b_sb[:, bass.ds(w * (BPW * 8) + 8 * j, 8)],
                            op=mybir.AluOpType.add)
                        oh = sp.tile([128, 128], dt.bfloat16)
                        nc.vector.tensor_tensor(
                            out=oh[:],
                            in0=li32[:, bass.ds(w * BPW + j, 1)]
                                .to_broadcast([128, 128]),
                            in1=iota_oh[:],
                            op=mybir.AluOpType.is_equal)
                        nc.tensor.matmul(hp[:], oh[:], msgb[:],
                                         start=(j == 0), stop=(j == BPW - 1))
                    if it == 0:
                        hw1 = sp.tile([128, 8], dt.bfloat16)
                        nc.scalar.activation(hw1[:], hp[:], AF.Copy,
                                             scale=SCALE)
                        nc.sync.dma_start(H1d[bass.ds(w * 128, 128), :],
                                          hw1[:])
                    else:
                        # o0 rows 32:40 <- H2 (feature-major)
                        hw1f = sp.tile([128, 8], dt.bfloat16)
                        nc.scalar.activation(hw1f[:], hp[:], AF.Copy)
                        tph = psh.tile([128, 128], dt.bfloat16, tag="tph")
                        nc.tensor.transpose(tph[0:8, :], hw1f[:], ident[:])
                        nc.scalar.activation(o0h[:, bass.ts(w, 128)],
                                             tph[0:8, :], AF.Copy)
                        # o0 rows 0:32 <- node features of this core's shard
                        ftw = sp.tile([128, 32], dt.bfloat16, tag="ftw")
                        nc.sync.dma_start(ftw[:],
                                          ft_d[bass.ds(w * 128, 128), :])
                        tpf = psh.tile([128, 128], dt.bfloat16, tag="tpf")
                        nc.tensor.transpose(tpf[0:32, :], ftw[:], ident[:])
                        nc.scalar.activation(o0f[:, bass.ts(w, 128)],
                                             tpf[0:32, :], AF.Copy)
                if it == 0:
                    nc.gpsimd.collective_compute(
                        "AllGather", OP.bypass, replica_groups=grp,
                        ins=[H1d[:, :].opt()], outs=[H1F[:, :].opt()])
                else:
                    nc.sync.dma_start(o0_d[0:32, :], o0f[:])
                    nc.sync.dma_start(o0_d[32:40, :], o0h[:])

    # ---------------- R1-R10: readout BN iterations ---------------------
    if PH < 6:
        return _finish_stub(nc, tile, mybir, o3_d, VSH)
    CHK = 448
    NCHK = VSH // CHK
    assert NCHK * CHK == VSH
    NPAD = VP - V   # pad columns, all on the last core
    for r in range(10):
        with tile.TileContext(nc) as tc:
            with (
                tc.tile_pool(name=f"r{r}", bufs=1) as res,
                tc.tile_pool(name=f"r{r}s", bufs=3) as sp,
                tc.tile_pool(name=f"r{r}p", bufs=2, space="PSUM") as psp,
                tc.tile_pool(name=f"r{r}q", bufs=2, space="PSUM") as psq,
            ):
                o = res.tile([40, VSH], dt.float32)
                o0t = res.tile([40, VSH], dt.float32)
                sqt = res.tile([40, VSH], dt.float32)
                wro = res.tile([40, 96], dt.float32)
                brd = res.tile([40, 8], dt.float32)
                i40 = res.tile([40, 40], dt.float32)
                opad = res.tile([40, 1], dt.float32)
                ident = res.tile([128, 128], dt.bfloat16)
                nc.sync.dma_start(o[:], o0_d[:, :] if r == 0 else od_d[:, :])
                nc.sync.dma_start(o0t[:], o0_d[:, :])
                nc.sync.dma_start(wro[:], wro_d[:, :])
                nc.sync.dma_start(brd[:], brd_d[:, :])
                make_identity(nc, ident[:])
                nc.vector.tensor_copy(i40[:], ident[0:40, 0:40])
                if r == 0:
                    nc.vector.memset(opad[:], 0.0)
                else:
                    nc.sync.dma_start(opad[:], opad_d[:, :])
                with tc.For_i(0, NCHK, 1) as i:
                    cs = bass.ts(i, CHK)
                    ps = psp.tile([40, CHK], dt.float32)
                    nc.tensor.matmul(ps[:], wro[0:40, 0:40], o[:, cs],
                                     start=True, stop=True)
                    t1 = sp.tile([40, CHK], dt.float32)
                    nc.scalar.activation(t1[:], ps[:], AF.Prelu,
                                         bias=brd[0:40, 0:1], scale=1.0,
                                         alpha=0.25)
                    ps2 = psp.tile([40, CHK], dt.float32)
                    nc.tensor.matmul(ps2[:], wro[0:40, 40:80], t1[:],
                                     start=True, stop=False)
                    nc.tensor.matmul(ps2[:], i40[:], o0t[:, cs],
                                     start=False, stop=True)
                    nc.scalar.activation(o[:, cs], ps2[:], AF.Prelu,
                                         bias=brd[0:40, 1:2], scale=1.0,
                                         alpha=0.25)
                # pad-column chain (zero input, no residual)
                psa = psq.tile([40, 1], dt.float32)
                nc.tensor.matmul(psa[:], wro[0:40, 0:40], opad[:],
                                 start=True, stop=True)
                t1p = sp.tile([40, 1], dt.float32, tag="t1p")
                nc.scalar.activation(t1p[:], psa[:], AF.Prelu,
                                     bias=brd[0:40, 0:1], scale=1.0,
                                     alpha=0.25)
                psb = psq.tile([40, 1], dt.float32)
                nc.tensor.matmul(psb[:], wro[0:40, 40:80], t1p[:],
                                 start=True, stop=True)
                nc.scalar.activation(opad[:], psb[:], AF.Prelu,
                                     bias=brd[0:40, 1:2], scale=1.0,
                                     alpha=0.25)
                # stats + AllReduce
                arin = res.tile([40, 2], dt.float32)
                nc.vector.tensor_reduce(out=arin[:, 0:1], in_=o[:],
                                        axis=mybir.AxisListType.X,
                                        op=OP.add)
                nc.scalar.activation(sqt[:], o[:], AF.Square)
                nc.vector.tensor_reduce(out=arin[:, 1:2], in_=sqt[:],
                                        axis=mybir.AxisListType.X,
                                        op=OP.add)
                ars = res.tile([40, 2], dt.float32)
                if os.environ.get("K2_NOAR"):
                    nc.vector.tensor_copy(ars[:], arin[:])
                else:
                    nc.sync.dma_start(arb_d[:, :], arin[:])
                    nc.gpsimd.collective_compute(
                        "AllReduce", OP.add, replica_groups=grp,
                        ins=[arb_d[:, :].opt()], outs=[arB_d[:, :].opt()])
                    nc.sync.dma_start(ars[:], arB_d[:, :])
                # corrections for NPAD all-zero-input pad columns
                tca = res.tile([40, 8], dt.float32)
                nc.scalar.activation(tca[:, 0:1], opad[:], AF.Copy,
                                     scale=float(NPAD))
                nc.vector.tensor_tensor(out=tca[:, 1:2], in0=ars[:, 0:1],
                                        in1=tca[:, 0:1], op=OP.subtract)
                nc.scalar.activation(tca[:, 2:3], opad[:], AF.Square)
                nc.scalar.activation(tca[:, 3:4], tca[:, 2:3], AF.Copy,
                                     scale=float(NPAD))
                nc.vector.tensor_tensor(out=tca[:, 4:5], in0=ars[:, 1:2],
                                        in1=tca[:, 3:4], op=OP.subtract)
                # mean / var / rstd
                mean = res.tile([40, 4], dt.float32)
                nc.scalar.activation(mean[:, 0:1], tca[:, 1:2], AF.Copy,
                                     scale=1.0 / V)
                nc.scalar.activation(mean[:, 1:2], tca[:, 4:5], AF.Copy,
                                     scale=1.0 / V)
                nc.scalar.activation(mean[:, 2:3], mean[:, 0:1], AF.Square)
                nc.vector.tensor_tensor(out=mean[:, 3:4], in0=mean[:, 1:2],
                                        in1=mean[:, 2:3], op=OP.subtract)
                veps = res.tile([40, 4], dt.float32)
                nc.vector.tensor_scalar_add(veps[:, 0:1], mean[:, 3:4], EPS)
                nc.vector.reciprocal(veps[:, 1:2], veps[:, 0:1])
                nc.scalar.sqrt(veps[:, 2:3], veps[:, 1:2])
                sc = res.tile([40, 4], dt.float32)
                nc.vector.tensor_tensor(out=sc[:, 0:1], in0=veps[:, 2:3],
                                        in1=brd[0:40, 2:3], op=OP.mult)
                nc.vector.tensor_tensor(out=sc[:, 1:2], in0=mean[:, 0:1],
                                        in1=sc[:, 0:1], op=OP.mult)
                nc.vector.tensor_tensor(out=sc[:, 2:3], in0=brd[0:40, 3:4],
                                        in1=sc[:, 1:2], op=OP.subtract)
                nc.scalar.activation(o[:], o[:], AF.Identity,
                                     bias=sc[:, 2:3], scale=sc[:, 0:1])
                nc.scalar.activation(opad[:], opad[:], AF.Identity,
                                     bias=sc[:, 2:3], scale=sc[:, 0:1])
                nc.sync.dma_start(od_d[:, :], o[:])
                nc.sync.dma_start(opad_d[:, :], opad[:])

    # ---------------- R11: l2 head + l3 tail -> o3 ----------------------
    with tile.TileContext(nc) as tc:
        with (
            tc.tile_pool(name="r11", bufs=1) as res,
            tc.tile_pool(name="r11s", bufs=3) as sp,
            tc.tile_pool(name="r11p", bufs=2, space="PSUM") as psp,
        ):
            o = res.tile([40, VSH], dt.float32)
            o3sb = res.tile([2, VSH], dt.bfloat16)
            wro = res.tile([40, 96], dt.float32)
            brd = res.tile([40, 8], dt.float32)
            i40 = res.tile([40, 40], dt.float32)
            ident = res.tile([128, 128], dt.bfloat16)
            nc.sync.dma_start(o[:], od_d[:, :])
            nc.sync.dma_start(wro[:], wro_d[:, :])
            nc.sync.dma_start(brd[:], brd_d[:, :])
            make_identity(nc, ident[:])
            nc.vector.tensor_copy(i40[:], ident[0:40, 0:40])
            with tc.For_i(0, NCHK, 1) as i:
                cs = bass.ts(i, CHK)
                ps = psp.tile([40, CHK], dt.float32)
                nc.tensor.matmul(ps[0:2, :], wro[0:40, 80:82], o[:, cs],
                                 start=True, stop=True)
                o2c = sp.tile([2, CHK], dt.float32, tag="o2c")
                nc.scalar.activation(o2c[:], ps[0:2, :], AF.Prelu,
                                     bias=brd[0:2, 4:5], scale=1.0,
                                     alpha=0.25)
                o3c = o2c
                for l in range(10):
                    psl = psp.tile([40, CHK], dt.float32)
                    nc.tensor.matmul(psl[0:2, :], wro[0:2, 82:84], o3c[:],
                                     start=True, stop=True)
                    t3 = sp.tile([2, CHK], dt.float32, tag="t3")
                    nc.scalar.activation(t3[:], psl[0:2, :], AF.Prelu,
                                         bias=brd[0:2, 5:6], scale=1.0,
                                         alpha=0.25)
                    psl2 = psp.tile([40, CHK], dt.float32)
                    nc.tensor.matmul(psl2[0:2, :], wro[0:2, 84:86], t3[:],
                                     start=True, stop=False)
                    nc.tensor.matmul(psl2[0:2, :], i40[0:2, 0:2], o2c[:],
                                     start=False, stop=True)
                    o3n = sp.tile([2, CHK], dt.float32, tag="o3n")
                    nc.scalar.activation(o3n[:], psl2[0:2, :], AF.Prelu,
                                         bias=brd[0:2, 6:7], scale=1.0,
                                         alpha=0.25)
                    o3c = o3n
                nc.vector.tensor_copy(o3sb[:, cs], o3c[:])
            nc.sync.dma_start(o3_d[:, :], o3sb[:])

    nc.compile()
    return nc


def _finish_stub(nc, tile, mybir, o3_d, VSH):
    dt = mybir.dt
    with tile.TileContext(nc) as tc:
        with tc.tile_pool(name="stub", bufs=1) as p:
            t = p.tile([2, VSH], dt.bfloat16)
            nc.vector.memset(t[:], 0.0)
            nc.sync.dma_start(o3_d[:, :], t[:])
    nc.compile()
    return nc


def _prelu(x, a):
    return np.where(x >= 0, x, a * x)


_PACK_CACHE = {}


def _host_pack(X_Node, X_Neis):
    import hashlib
    hkey = hashlib.blake2b(X_Node.tobytes(), digest_size=16)
    hkey.update(X_Neis.tobytes())
    hkey = hkey.digest()
    hit = _PACK_CACHE.get(hkey)
    if hit is not None:
        return hit
    xn = X_Node.astype(np.int32)
    win = xn >> 7
    counts = np.bincount(win, minlength=NW)
    BPW = max(9, int(-(-int(counts.max()) // 128)))
    NBLK4 = -(-(NWC * BPW) // 4) * 4
    order = np.argsort(win, kind="stable").astype(np.int32)
    starts = np.zeros(NW + 1, np.int32)
    np.cumsum(counts, out=starts[1:])
    ws = win[order]
    r = np.arange(E, dtype=np.int32) - starts[ws]
    core = ws // NWC
    bcol = (ws % NWC) * BPW + (r >> 7)
    prow = r & 127
    li8 = np.full((NCORES, 128, NBLK4), 128, np.uint8)
    ixe16 = np.zeros((NCORES, 128, NBLK4), np.uint16)
    li8[core, prow, bcol] = (xn[order] & 127).astype(np.uint8)
    ixe16[core, prow, bcol] = X_Neis[order].astype(np.uint16)
    _PACK_CACHE[hkey] = (BPW, li8, ixe16)
    return BPW, li8, ixe16


def kernel(**inputs):
    X_Node = np.asarray(inputs["X_Node"]).astype(np.int64)
    X_Neis = np.asarray(inputs["X_Neis"]).astype(np.int64)
    fM = np.asarray(inputs["feature_Matrix"], dtype=np.float32)
    H0 = np.asarray(inputs["node_states"], dtype=np.float32)
    g = {k: np.asarray(v, dtype=np.float32) for k, v in inputs.items()
         if k not in ("X_Node", "X_Neis")}

    BPW, li8, ixe16 = _host_pack(X_Node, X_Neis)

    ftab = np.zeros((VP, 32), BF16)
    ftab[:V] = fM.T.astype(BF16)
    h0tab = np.zeros((VP, 8), BF16)
    h0tab[:V] = (H0 * SCALE).astype(BF16)

    wfull = np.zeros((WR, 64), BF16)
    for k, wname in enumerate(["xi1w", "xi2w", "xi3w", "xi3aw"]):
        wfull[64 * k:64 * k + 64] = g[wname].T.astype(BF16)
    wfull[256:288, 0:32] = g["r1w"].T.astype(BF16)
    wfull[288:320, 0:8] = g["r2w"].T.astype(BF16)
    wfull[320:328, 0:8] = g["r3aw"].T.astype(BF16)

    bxv = np.zeros((128, 8), np.float32)
    for i, bn in enumerate(["xi1b", "xi2b", "xi3b", "xi3ab"]):
        bxv[0:64, i] = g[bn]
        bxv[64:128, i] = g[bn]
    bxv[:, 4] = np.tile(g["r1b"], 4)
    bxv[0:32, 5] = np.tile(g["r2b"], 4)
    bxv[0:32, 6] = np.tile(g["r3ab"], 4)

    wro = np.zeros((40, 96), np.float32)
    wro[0:40, 0:40] = g["l1w"].T
    wro[0:40, 40:80] = g["l1aw"].T
    wro[0:40, 80:82] = g["l2w"].T
    wro[0:2, 82:84] = g["l3w"].T
    wro[0:2, 84:86] = g["l3aw"].T
    brd = np.zeros((40, 8), np.float32)
    brd[0:40, 0] = g["l1b"]
    brd[0:40, 1] = g["l1ab"]
    brd[0:40, 2] = g["bn_g"]
    brd[0:40, 3] = g["bn_b"]
    brd[0:2, 4] = g["l2b"]
    brd[0:2, 5] = g["l3b"]
    brd[0:2, 6] = g["l3ab"]

    in_maps = []
    for c in range(NCORES):
        in_maps.append({
            "ftsh": ftab[VSH * c:VSH * (c + 1)],
            "h0sh": h0tab[VSH * c:VSH * (c + 1)],
            "wsh": wfull[WSH * c:WSH * (c + 1)],
            "li8": li8[c], "ixe": ixe16[c], "bx": bxv,
            "wro": wro, "brd": brd,
        })

    PH = int(os.environ.get("K2_PHASES", "99"))
    key = ("nc", BPW, PH, os.environ.get("K2_NOAR", ""))
    if key not in _CACHE:
        _CACHE[key] = _build_nc(BPW, PH)
    nc = _CACHE[key]

    t0 = _time.time()
    res = bass_utils.run_bass_kernel_spmd(
        nc, in_maps, core_ids=list(range(NCORES)), trace=False)
    LAST_RESULT["run_wall_s"] = _time.time() - t0
    LAST_RESULT["exec_time_ns"] = res.exec_time_ns

    o3f = np.concatenate([res.results[c]["o3f"] for c in range(NCORES)],
                         axis=1)[:, :V]
    return np.concatenate([o3f[0], o3f[1]], axis=0).astype(np.float32)
